# revision 30
# baseline (speedup 1.0000x reference)
"""3x3 same-conv (NHWC, 32x56x56x128 -> 32x56x56x256) + bias + ReLU on 8 TRN2 cores.

Strategy: data-parallel over batch (4 images/core). Per core, the conv is
9 shifted matmuls accumulated in PSUM with Cin=128 as the contraction dim
over a PACKED 56-wide slab: XpT[cin, p] with p(r,c) = (r+1)*56+c+1, only
vertical pad rows (no left/right pad columns). The anchor axis is then a
dense [57, 3193) range tiled in 25 windows of exactly 128 anchors: every
matmul runs with the full 128 PSUM partitions / 128-column stationary
operand (fast weight load), there are zero junk anchors, and the store is
a handful of large regular DMAs per image ([128, nw, 256] window-major ->
pixel-major DRAM).

The packed layout makes the horizontal taps WRAP at row edges, so output
columns 0 and 55 are recomputed by a small edge pass: four column strips
(cols 0,55,1,54) are copied into a column-major mini-slab [cin, 4*58],
12 matmuls of M=56 rebuild the two edge columns exactly, and their stores
overwrite the wrapped values (DRAM write-after-write order is enforced by
the tile framework's shadow-memory deps plus SWDGE queue FIFO).

Input transposes run on the PE (identity-matmul of 2-row stage chunks;
the DMA-XBAR alternative measures ~1.2us/chunk and serializes against
other DMAs). A warm-up burst of identity transposes at t=0 releases the
HAM clock gate before the first real matmul. Input stage loads are
casting SWDGE DMAs (fp32->fp16) issued up front.
"""

import os
from contextlib import ExitStack

import numpy as np

import concourse.bass as bass
import concourse.bacc as bacc
import concourse.mybir as mybir
import concourse.tile as tile
from concourse.bass_utils import run_bass_kernel_spmd
from concourse.masks import make_identity

N_CORES = 8
B, H, W, CIN, COUT = 32, 56, 56, 128, 256
BPC = B // N_CORES            # images per core
PIX = H * W                   # 3136
SLAB_W = 3328                 # 1 + pad row + 56 rows + pad row + slop
ABASE = W + 1                 # first anchor (pixel (0,0) at 57)
NW = 25                       # 128-anchor windows per image
WM = 128                      # anchors per window
RPC = 2                       # image rows per transpose chunk
CHUNK_PIX = RPC * W           # 112
NCHUNK = H // RPC             # 28
ES = H + 2                    # edge strip length (58)

TAP_OFFS = [(dh - 1) * W + (dw - 1) for dh in range(3) for dw in range(3)]
F32 = mybir.dt.float32
F16 = mybir.dt.float16

LAST_RESULTS = None


def _build(with_bias: bool):
    nc = bacc.Bacc("TRN2", target_bir_lowering=False, debug=False)
    x_h = nc.declare_dram_parameter("prev_a", [BPC, H, W, CIN], F32, isOutput=False)
    w_h = nc.declare_dram_parameter("filter_w", [3, 3, CIN, COUT], F32, isOutput=False)
    b_h = nc.declare_dram_parameter("filter_b", [1, 1, 1, COUT], F32, isOutput=False)
    y_h = nc.declare_dram_parameter("out", [BPC, H, W, COUT], F32, isOutput=True)
    x_ap, w_ap, b_ap, y_ap = x_h.ap(), w_h.ap(), b_h.ap(), y_h.ap()

    with tile.TileContext(nc) as tc, ExitStack() as ctx:
        const_pool = ctx.enter_context(tc.tile_pool(name="const", bufs=1))
        xslab_pool = ctx.enter_context(tc.tile_pool(name="xslab", bufs=1))
        stage_pool = ctx.enter_context(tc.tile_pool(name="stage", bufs=1))
        edge_pool = ctx.enter_context(tc.tile_pool(name="edge", bufs=2))
        out_pool = ctx.enter_context(tc.tile_pool(name="outsb", bufs=2))
        psum_mm = ctx.enter_context(
            tc.tile_pool(name="psmm", bufs=3, space=bass.MemorySpace.PSUM)
        )
        psum_tp = ctx.enter_context(
            tc.tile_pool(name="pstp", bufs=3, space=bass.MemorySpace.PSUM)
        )
        psum_ed = ctx.enter_context(
            tc.tile_pool(name="psed", bufs=2, space=bass.MemorySpace.PSUM)
        )

        identity = const_pool.tile([CHUNK_PIX, CHUNK_PIX], F16, tag="ident")
        make_identity(nc, identity[:])

        # Stage tiles + loads: all four images' [pix, cin] fp16 stages up
        # front (casting SWDGE DMAs); the weight cast rides between the first
        # image's chunks so the first window can start ASAP.
        stages = [
            stage_pool.tile(
                [CHUNK_PIX, NCHUNK * CIN], F16, tag=f"stg{i}", name=f"stg{i}"
            )
            for i in range(BPC)
        ]
        wslab = const_pool.tile([CIN, 9 * COUT], F16, tag="wslab")

        def emit_load(i, c0, c1):
            src = (
                x_ap[i]
                .rearrange("h w c -> (h w) c")
                .rearrange("(n p) c -> n p c", p=CHUNK_PIX)
                .transpose([1, 0, 2])
            )
            dstv = stages[i][:].rearrange("p (n c) -> p n c", n=NCHUNK)
            nc.gpsimd.dma_start(out=dstv[:, c0:c1, :], in_=src[:, c0:c1, :])

        # Weights ride the (otherwise idle) HWDGE queue as fp32 + DVE cast;
        # a casting SWDGE load here would serialize behind the stage loads
        # and gate the first conv matmul (~8us later, measured).
        # Weights: 9 per-tap fp32 DMAs alternating across BOTH HWDGE rings
        # (each ring moves only ~150 GB/s, so a single 1.2MB transfer gates
        # the first conv matmul by ~5us), cast tap-by-tap on DVE so the
        # first window can consume taps as they stream in.
        wstage = const_pool.tile([CIN, 9 * COUT], F32, tag="wstage")
        wsrc = w_ap.rearrange("a b k n -> (a b) k n").transpose([1, 0, 2])
        wdst = wstage[:].rearrange("k (t n) -> k t n", t=9)
        for t in range(9):
            eng = nc.sync if t % 2 == 0 else nc.scalar
            eng.dma_start(out=wdst[:, t : t + 1, :], in_=wsrc[:, t : t + 1, :])
            nc.vector.tensor_copy(
                wslab[:, t * COUT : (t + 1) * COUT],
                wstage[:, t * COUT : (t + 1) * COUT],
            )

        emit_load(0, 0, 2)
        emit_load(0, 2, 7)
        emit_load(0, 7, 14)
        emit_load(0, 14, 21)
        emit_load(0, 21, 28)
        for i in range(1, BPC):
            for c0 in range(0, NCHUNK, 7):
                emit_load(i, c0, c0 + 7)

        if with_bias:
            bias_st = const_pool.tile([1, COUT], F32, tag="bias_st")
            nc.sync.dma_start(
                out=bias_st[:], in_=b_ap.rearrange("a b c n -> (a b c) n")
            )
            bias_sb = const_pool.tile([1, COUT], F16, tag="bias")
            nc.vector.tensor_copy(bias_sb[:], bias_st[:])
            ones_sb = const_pool.tile([1, 128], F16, tag="ones")
            nc.gpsimd.memset(ones_sb[:], 1.0)

        # PE warm-up: ~30 junk identity transposes keep the PE busy through
        # the HAM activity window (~3.4us) while the first DMAs land, so the
        # first real matmul runs at the warm 2.4 GHz clock.
        for _ in range(12):
            pwu = psum_tp.tile([CIN, CHUNK_PIX], F16, tag="pst")
            nc.tensor.transpose(
                pwu[0:CHUNK_PIX, 0:CHUNK_PIX], identity[:], identity[:]
            )

        # Per-image packed transposed slabs [cin, 1 + 58*56 rows + slop]
        xslabs = []
        for i in range(BPC):
            sl = xslab_pool.tile([CIN, SLAB_W], F16, tag=f"xs{i}")
            xslabs.append(sl)
            nc.vector.memset(sl[:, 0 : ABASE], 0.0)           # lead + top pad row
            nc.vector.memset(sl[:, (H + 1) * W + 1 : SLAB_W], 0.0)  # bottom pad + slop

        def emit_transpose(i, cidx):
            # stage [112, cin] chunk -> PE transpose -> PSUM [cin, 112] ->
            # ACT copy into the packed slab (contiguous 112 span)
            pst = psum_tp.tile([CIN, CHUNK_PIX], F16, tag="pst")
            nc.tensor.transpose(
                pst[:], stages[i][:, cidx * CIN : (cidx + 1) * CIN], identity[:]
            )
            d0 = (RPC * cidx + 1) * W + 1
            nc.scalar.activation(
                xslabs[i][:, d0 : d0 + CHUNK_PIX],
                pst[:],
                mybir.ActivationFunctionType.Copy,
            )

        # edge mini-slab: three 115-wide regions [pad a(56) sharedpad b(56)
        # pad] = A:[col0|col55] B:[col1|zeros] C:[zeros|col54], so all nine
        # edge matmuls are FULL M=113 windows (partitions 0-55 = left col,
        # 57-112 = right col; the zero halves contribute nothing) -- no
        # partition-offset PSUM writes, which measure wrong on HW.
        RA, RB, RC = 0, 115, 230
        EW = 352
        EM = 113
        ECOPY = [(RA + 1, 0), (RA + 58, 55), (RB + 1, 1), (RC + 58, 54)]

        def emit_ebuild(i):
            ed = edge_pool.tile([CIN, EW], F16, tag="E")
            nc.vector.memset(ed[:], 0.0)
            for base, col in ECOPY:
                src = (
                    xslabs[i][:, W + 1 + col : W + 1 + col + H * W]
                    .rearrange("p (r c) -> p r c", c=W)[:, :, 0:1]
                    .rearrange("p r c -> p (r c)")
                )
                nc.scalar.activation(
                    ed[:, base : base + H],
                    src,
                    mybir.ActivationFunctionType.Copy,
                )
            return ed

        def emit_window(i, w, oslab):
            q0 = ABASE + WM * w
            ps = psum_mm.tile([WM, COUT], F32, tag="psmm")
            for t in range(9):
                w0 = q0 + TAP_OFFS[t]
                nc.tensor.matmul(
                    ps[:],
                    xslabs[i][:, w0 : w0 + WM],
                    wslab[:, t * COUT : (t + 1) * COUT],
                    start=(t == 0),
                    stop=(t == 8 and not with_bias),
                )
            if with_bias:
                nc.tensor.matmul(
                    ps[:], ones_sb[:1, :WM], bias_sb[:1, :], start=False, stop=True
                )
            nc.vector.tensor_scalar_max(
                oslab[:, w * COUT : (w + 1) * COUT], ps[:], 0.0
            )

        # main-store chunk boundaries (after these windows' relu); finer at
        # the image end so the final transfer (which the edge stores
        # WAW-wait on) is small
        STORE_AT = {
            6: (0, 7),
            13: (7, 14),
            19: (14, 20),
            20: (20, 21),
            21: (21, 22),
            22: (22, 23),
            23: (23, 24),
        }

        def emit_store_chunk(i, oslab, w0, w1):
            dst = (
                y_ap[i]
                .rearrange("h w c -> (h w) c")[w0 * WM : w1 * WM, :]
                .rearrange("(w p) c -> p w c", p=WM)
            )
            src = oslab[:, w0 * COUT : w1 * COUT].rearrange(
                "p (w k) -> p w k", k=COUT
            )
            nc.gpsimd.dma_start(out=dst, in_=src)

        def emit_store_last(i, oslab):
            n = PIX - 24 * WM  # 64
            dst = y_ap[i].rearrange("h w c -> (h w) c")[24 * WM :, :]
            nc.gpsimd.dma_start(out=dst, in_=oslab[0:n, 24 * COUT : 25 * COUT])

        # edge pass: 9 full-window matmuls rebuild output cols 0 / 55
        # exactly; their stores overwrite the wrapped main-store values and
        # MUST sit behind the image's main stores on the same SWDGE queue --
        # FIFO ring order is the only cross-DMA write-ordering guarantee
        # (stores on a different queue measured racy: ~1-2 columns stale).
        E_REGION = {0: RC, 1: RA, 2: RB}  # dw -> region base

        def emit_edge(i, ed):
            pe = psum_ed.tile([EM, COUT], F32, tag="psed")
            k = 0
            nmm = 9 + (1 if with_bias else 0)
            for dh in range(3):
                for dw in range(3):
                    t = dh * 3 + dw
                    base = E_REGION[dw]
                    nc.tensor.matmul(
                        pe[:],
                        ed[:, base + dh : base + dh + EM],
                        wslab[:, t * COUT : (t + 1) * COUT],
                        start=(k == 0),
                        stop=(k == nmm - 1),
                    )
                    k += 1
            if with_bias:
                nc.tensor.matmul(
                    pe[:], ones_sb[:1, :EM], bias_sb[:1, :], start=False, stop=True
                )
            esb = edge_pool.tile([EM, COUT], F32, tag="esb")
            nc.vector.tensor_scalar_max(esb[:], pe[:], 0.0)
            return esb

        def emit_edge_stores(i, esb):
            nc.gpsimd.dma_start(out=y_ap[i][:, 0, :], in_=esb[0:H, :])
            nc.gpsimd.dma_start(out=y_ap[i][:, 55, :], in_=esb[H + 1 : H + 1 + H, :])

        # Image 0's transposes up front; image i+1's are interleaved between
        # image i's windows so the PE never waits on a bulk transpose phase.
        for c in range(NCHUNK):
            emit_transpose(0, c)
        ed = emit_ebuild(0)
        for i in range(BPC):
            oslab = out_pool.tile([WM, NW * COUT], F32, tag="osb")
            done = 0
            for w in range(NW):
                emit_window(i, w, oslab)
                if w in STORE_AT:
                    emit_store_chunk(i, oslab, *STORE_AT[w])
                if w == 21:
                    esb = emit_edge(i, ed)
                if i + 1 < BPC:
                    want = (w + 1) * NCHUNK // NW
                    while done < want:
                        emit_transpose(i + 1, done)
                        done += 1
            emit_store_last(i, oslab)
            emit_edge_stores(i, esb)
            if i + 1 < BPC:
                while done < NCHUNK:
                    emit_transpose(i + 1, done)
                    done += 1
            ed = emit_ebuild(i + 1) if i + 1 < BPC else None

    nc.compile()
    return nc


_CACHE = {}


def _get_nc(with_bias: bool):
    if with_bias not in _CACHE:
        _CACHE[with_bias] = _build(with_bias)
    return _CACHE[with_bias]


def kernel(prev_a, filter_w, filter_b):
    global LAST_RESULTS
    prev_a = np.ascontiguousarray(prev_a, dtype=np.float32)
    filter_w = np.ascontiguousarray(filter_w, dtype=np.float32)
    filter_b = np.ascontiguousarray(filter_b, dtype=np.float32).reshape(1, 1, 1, COUT)
    with_bias = bool(np.any(filter_b))
    nc = _get_nc(with_bias)
    in_maps = [
        {
            "prev_a": prev_a[c * BPC : (c + 1) * BPC],
            "filter_w": filter_w,
            "filter_b": filter_b,
        }
        for c in range(N_CORES)
    ]
    trace = os.environ.get("KERNEL_TRACE") == "1"
    res = run_bass_kernel_spmd(nc, in_maps, list(range(N_CORES)), trace=trace)
    LAST_RESULTS = res
    return np.concatenate([res.results[c]["out"] for c in range(N_CORES)], axis=0)


# revision 33
# speedup vs baseline: 1.0070x; 1.0070x over previous
"""3x3 same-conv (NHWC, 32x56x56x128 -> 32x56x56x256) + bias + ReLU on 8 TRN2 cores.

Strategy: data-parallel over batch (4 images/core). Per core, the conv is
9 shifted matmuls accumulated in PSUM with Cin=128 as the contraction dim
over a PACKED 56-wide slab: XpT[cin, p] with p(r,c) = (r+1)*56+c+1, only
vertical pad rows (no left/right pad columns). The anchor axis is then a
dense [57, 3193) range tiled in 25 windows of exactly 128 anchors: every
matmul runs with the full 128 PSUM partitions / 128-column stationary
operand (fast weight load), there are zero junk anchors, and the store is
a handful of large regular DMAs per image ([128, nw, 256] window-major ->
pixel-major DRAM).

The packed layout makes the horizontal taps WRAP at row edges, so output
columns 0 and 55 are recomputed by a small edge pass: column strips are
copied into a column-major mini-slab of three zero-padded regions
([col0|col55] / [col1|zeros] / [zeros|col54]) so NINE full-window M=113
matmuls rebuild both edge columns exactly (no partition-offset PSUM
writes -- those land wrong on HW). The edge stores sit behind the
image's main stores on the same SWDGE ring: FIFO order is the only
cross-DMA write-ordering guarantee (a different queue measured racy).

Input transposes run on the PE (identity-matmul of 2-row stage chunks;
the DMA-XBAR alternative measures ~1.2us/chunk and serializes against
other DMAs). A warm-up burst of identity transposes at t=0 releases the
HAM clock gate before the first real matmul. Input stage loads are
casting SWDGE DMAs (fp32->fp16) issued up front, except the weights and
first stage chunks which ride the faster HWDGE ring as fp32 + DVE cast.
"""

import os
from contextlib import ExitStack

import numpy as np

import concourse.bass as bass
import concourse.bacc as bacc
import concourse.mybir as mybir
import concourse.tile as tile
from concourse.bass_utils import run_bass_kernel_spmd
from concourse.masks import make_identity

N_CORES = 8
B, H, W, CIN, COUT = 32, 56, 56, 128, 256
BPC = B // N_CORES            # images per core
PIX = H * W                   # 3136
SLAB_W = 3328                 # 1 + pad row + 56 rows + pad row + slop
ABASE = W + 1                 # first anchor (pixel (0,0) at 57)
NW = 25                       # 128-anchor windows per image
WM = 128                      # anchors per window
RPC = 2                       # image rows per transpose chunk
CHUNK_PIX = RPC * W           # 112
NCHUNK = H // RPC             # 28
ES = H + 2                    # edge strip length (58)

TAP_OFFS = [(dh - 1) * W + (dw - 1) for dh in range(3) for dw in range(3)]
F32 = mybir.dt.float32
F16 = mybir.dt.float16

LAST_RESULTS = None


def _build(with_bias: bool):
    nc = bacc.Bacc("TRN2", target_bir_lowering=False, debug=False)
    x_h = nc.declare_dram_parameter("prev_a", [BPC, H, W, CIN], F32, isOutput=False)
    w_h = nc.declare_dram_parameter("filter_w", [3, 3, CIN, COUT], F32, isOutput=False)
    b_h = nc.declare_dram_parameter("filter_b", [1, 1, 1, COUT], F32, isOutput=False)
    y_h = nc.declare_dram_parameter("out", [BPC, H, W, COUT], F32, isOutput=True)
    x_ap, w_ap, b_ap, y_ap = x_h.ap(), w_h.ap(), b_h.ap(), y_h.ap()

    with tile.TileContext(nc) as tc, ExitStack() as ctx:
        const_pool = ctx.enter_context(tc.tile_pool(name="const", bufs=1))
        xslab_pool = ctx.enter_context(tc.tile_pool(name="xslab", bufs=1))
        stage_pool = ctx.enter_context(tc.tile_pool(name="stage", bufs=1))
        edge_pool = ctx.enter_context(tc.tile_pool(name="edge", bufs=2))
        out_pool = ctx.enter_context(tc.tile_pool(name="outsb", bufs=2))
        psum_mm = ctx.enter_context(
            tc.tile_pool(name="psmm", bufs=3, space=bass.MemorySpace.PSUM)
        )
        psum_tp = ctx.enter_context(
            tc.tile_pool(name="pstp", bufs=3, space=bass.MemorySpace.PSUM)
        )
        psum_ed = ctx.enter_context(
            tc.tile_pool(name="psed", bufs=2, space=bass.MemorySpace.PSUM)
        )

        identity = const_pool.tile([CHUNK_PIX, CHUNK_PIX], F16, tag="ident")
        make_identity(nc, identity[:])

        # Stage tiles + loads: all four images' [pix, cin] fp16 stages up
        # front (casting SWDGE DMAs); the weight cast rides between the first
        # image's chunks so the first window can start ASAP.
        stages = [
            stage_pool.tile(
                [CHUNK_PIX, NCHUNK * CIN], F16, tag=f"stg{i}", name=f"stg{i}"
            )
            for i in range(BPC)
        ]
        wslab = const_pool.tile([CIN, 9 * COUT], F16, tag="wslab")

        def emit_load(i, c0, c1):
            src = (
                x_ap[i]
                .rearrange("h w c -> (h w) c")
                .rearrange("(n p) c -> n p c", p=CHUNK_PIX)
                .transpose([1, 0, 2])
            )
            dstv = stages[i][:].rearrange("p (n c) -> p n c", n=NCHUNK)
            nc.gpsimd.dma_start(out=dstv[:, c0:c1, :], in_=src[:, c0:c1, :])

        # Weights ride the (otherwise idle) HWDGE queue as fp32 + DVE cast;
        # a casting SWDGE load here would serialize behind the stage loads
        # and gate the first conv matmul (~8us later, measured).
        # Weights and the first stage chunks ride the HWDGE ring as fp32
        # (cast by DVE): the SWDGE casting transfers are too slow to feed
        # the PE early, and a single 1.2MB weights DMA would gate the first
        # conv matmul -- interleave them split in halves.
        wstage = const_pool.tile([CIN, 9 * COUT], F32, tag="wstage")
        wsrc = w_ap.rearrange("a b k n -> (a b) k n").transpose([1, 0, 2])
        wdst = wstage[:].rearrange("k (t n) -> k t n", t=9)
        HS = 6
        stage32 = const_pool.tile([CHUNK_PIX, HS * CIN], F32, tag="stage32")
        hs_src = (
            x_ap[0]
            .rearrange("h w c -> (h w) c")
            .rearrange("(n p) c -> n p c", p=CHUNK_PIX)
            .transpose([1, 0, 2])
        )
        hs_dst = stage32[:].rearrange("p (n c) -> p n c", n=HS)
        st_dst = stages[0][:].rearrange("p (n c) -> p n c", n=NCHUNK)
        nc.sync.dma_start(out=hs_dst[:, 0:2, :], in_=hs_src[:, 0:2, :])
        nc.vector.tensor_copy(st_dst[:, 0:2, :], hs_dst[:, 0:2, :])
        nc.sync.dma_start(out=wdst[:, 0:5, :], in_=wsrc[:, 0:5, :])
        nc.vector.tensor_copy(wslab[:, : 5 * COUT], wstage[:, : 5 * COUT])
        nc.sync.dma_start(out=hs_dst[:, 2:HS, :], in_=hs_src[:, 2:HS, :])
        nc.vector.tensor_copy(st_dst[:, 2:HS, :], hs_dst[:, 2:HS, :])
        nc.sync.dma_start(out=wdst[:, 5:9, :], in_=wsrc[:, 5:9, :])
        nc.vector.tensor_copy(wslab[:, 5 * COUT :], wstage[:, 5 * COUT :])

        emit_load(0, HS, 14)
        emit_load(0, 14, 21)
        emit_load(0, 21, 28)
        for i in range(1, BPC):
            for c0 in range(0, NCHUNK, 7):
                emit_load(i, c0, c0 + 7)

        if with_bias:
            bias_st = const_pool.tile([1, COUT], F32, tag="bias_st")
            nc.sync.dma_start(
                out=bias_st[:], in_=b_ap.rearrange("a b c n -> (a b c) n")
            )
            bias_sb = const_pool.tile([1, COUT], F16, tag="bias")
            nc.vector.tensor_copy(bias_sb[:], bias_st[:])
            ones_sb = const_pool.tile([1, 128], F16, tag="ones")
            nc.gpsimd.memset(ones_sb[:], 1.0)

        # PE warm-up: ~30 junk identity transposes keep the PE busy through
        # the HAM activity window (~3.4us) while the first DMAs land, so the
        # first real matmul runs at the warm 2.4 GHz clock.
        for _ in range(14):
            pwu = psum_tp.tile([CIN, CHUNK_PIX], F16, tag="pst")
            nc.tensor.transpose(
                pwu[0:CHUNK_PIX, 0:CHUNK_PIX], identity[:], identity[:]
            )

        # Per-image packed transposed slabs [cin, 1 + 58*56 rows + slop]
        xslabs = []
        for i in range(BPC):
            sl = xslab_pool.tile([CIN, SLAB_W], F16, tag=f"xs{i}")
            xslabs.append(sl)
            nc.vector.memset(sl[:, 0 : ABASE], 0.0)           # lead + top pad row
            nc.vector.memset(sl[:, (H + 1) * W + 1 : SLAB_W], 0.0)  # bottom pad + slop

        def emit_transpose(i, cidx):
            # stage [112, cin] chunk -> PE transpose -> PSUM [cin, 112] ->
            # ACT copy into the packed slab (contiguous 112 span)
            pst = psum_tp.tile([CIN, CHUNK_PIX], F16, tag="pst")
            nc.tensor.transpose(
                pst[:], stages[i][:, cidx * CIN : (cidx + 1) * CIN], identity[:]
            )
            d0 = (RPC * cidx + 1) * W + 1
            nc.scalar.activation(
                xslabs[i][:, d0 : d0 + CHUNK_PIX],
                pst[:],
                mybir.ActivationFunctionType.Copy,
            )

        # edge mini-slab: three 115-wide regions [pad a(56) sharedpad b(56)
        # pad] = A:[col0|col55] B:[col1|zeros] C:[zeros|col54], so all nine
        # edge matmuls are FULL M=113 windows (partitions 0-55 = left col,
        # 57-112 = right col; the zero halves contribute nothing) -- no
        # partition-offset PSUM writes, which measure wrong on HW.
        RA, RB, RC = 0, 115, 230
        EW = 352
        EM = 113
        ECOPY = [(RA + 1, 0), (RA + 58, 55), (RB + 1, 1), (RC + 58, 54)]

        def emit_ebuild(i):
            ed = edge_pool.tile([CIN, EW], F16, tag="E")
            nc.vector.memset(ed[:], 0.0)
            for base, col in ECOPY:
                src = (
                    xslabs[i][:, W + 1 + col : W + 1 + col + H * W]
                    .rearrange("p (r c) -> p r c", c=W)[:, :, 0:1]
                    .rearrange("p r c -> p (r c)")
                )
                nc.scalar.activation(
                    ed[:, base : base + H],
                    src,
                    mybir.ActivationFunctionType.Copy,
                )
            return ed

        def emit_window(i, w, oslab):
            q0 = ABASE + WM * w
            ps = psum_mm.tile([WM, COUT], F32, tag="psmm")
            for t in range(9):
                w0 = q0 + TAP_OFFS[t]
                nc.tensor.matmul(
                    ps[:],
                    xslabs[i][:, w0 : w0 + WM],
                    wslab[:, t * COUT : (t + 1) * COUT],
                    start=(t == 0),
                    stop=(t == 8 and not with_bias),
                )
            if with_bias:
                nc.tensor.matmul(
                    ps[:], ones_sb[:1, :WM], bias_sb[:1, :], start=False, stop=True
                )
            nc.vector.tensor_scalar_max(
                oslab[:, w * COUT : (w + 1) * COUT], ps[:], 0.0
            )

        # main-store chunk boundaries (after these windows' relu); finer at
        # the image end so the final transfer (which the edge stores
        # WAW-wait on) is small
        STORE_AT = {
            6: (0, 7),
            13: (7, 14),
            19: (14, 20),
            20: (20, 21),
            21: (21, 22),
            22: (22, 23),
            23: (23, 24),
        }

        def emit_store_chunk(i, oslab, w0, w1):
            dst = (
                y_ap[i]
                .rearrange("h w c -> (h w) c")[w0 * WM : w1 * WM, :]
                .rearrange("(w p) c -> p w c", p=WM)
            )
            src = oslab[:, w0 * COUT : w1 * COUT].rearrange(
                "p (w k) -> p w k", k=COUT
            )
            nc.gpsimd.dma_start(out=dst, in_=src)

        def emit_store_last(i, oslab):
            n = PIX - 24 * WM  # 64
            dst = y_ap[i].rearrange("h w c -> (h w) c")[24 * WM :, :]
            nc.gpsimd.dma_start(out=dst, in_=oslab[0:n, 24 * COUT : 25 * COUT])

        # edge pass: 9 full-window matmuls rebuild output cols 0 / 55
        # exactly; their stores overwrite the wrapped main-store values and
        # MUST sit behind the image's main stores on the same SWDGE queue --
        # FIFO ring order is the only cross-DMA write-ordering guarantee
        # (stores on a different queue measured racy: ~1-2 columns stale).
        E_REGION = {0: RC, 1: RA, 2: RB}  # dw -> region base

        def emit_edge(i, ed):
            pe = psum_ed.tile([EM, COUT], F32, tag="psed")
            k = 0
            nmm = 9 + (1 if with_bias else 0)
            for dh in range(3):
                for dw in range(3):
                    t = dh * 3 + dw
                    base = E_REGION[dw]
                    nc.tensor.matmul(
                        pe[:],
                        ed[:, base + dh : base + dh + EM],
                        wslab[:, t * COUT : (t + 1) * COUT],
                        start=(k == 0),
                        stop=(k == nmm - 1),
                    )
                    k += 1
            if with_bias:
                nc.tensor.matmul(
                    pe[:], ones_sb[:1, :EM], bias_sb[:1, :], start=False, stop=True
                )
            esb = edge_pool.tile([EM, COUT], F32, tag="esb")
            nc.vector.tensor_scalar_max(esb[:], pe[:], 0.0)
            return esb

        def emit_edge_stores(i, esb):
            nc.gpsimd.dma_start(out=y_ap[i][:, 0, :], in_=esb[0:H, :])
            nc.gpsimd.dma_start(out=y_ap[i][:, 55, :], in_=esb[H + 1 : H + 1 + H, :])

        # Image 0's transposes up front; image i+1's are interleaved between
        # image i's windows so the PE never waits on a bulk transpose phase.
        for c in range(NCHUNK):
            emit_transpose(0, c)
        ed = emit_ebuild(0)
        for i in range(BPC):
            oslab = out_pool.tile([WM, NW * COUT], F32, tag="osb")
            done = 0
            for w in range(NW):
                emit_window(i, w, oslab)
                if w in STORE_AT:
                    emit_store_chunk(i, oslab, *STORE_AT[w])
                if w == 21:
                    esb = emit_edge(i, ed)
                if i + 1 < BPC:
                    want = (w + 1) * NCHUNK // NW
                    while done < want:
                        emit_transpose(i + 1, done)
                        done += 1
            emit_store_last(i, oslab)
            emit_edge_stores(i, esb)
            if i + 1 < BPC:
                while done < NCHUNK:
                    emit_transpose(i + 1, done)
                    done += 1
            ed = emit_ebuild(i + 1) if i + 1 < BPC else None

    nc.compile()
    return nc


_CACHE = {}


def _get_nc(with_bias: bool):
    if with_bias not in _CACHE:
        _CACHE[with_bias] = _build(with_bias)
    return _CACHE[with_bias]


def kernel(prev_a, filter_w, filter_b):
    global LAST_RESULTS
    prev_a = np.ascontiguousarray(prev_a, dtype=np.float32)
    filter_w = np.ascontiguousarray(filter_w, dtype=np.float32)
    filter_b = np.ascontiguousarray(filter_b, dtype=np.float32).reshape(1, 1, 1, COUT)
    with_bias = bool(np.any(filter_b))
    nc = _get_nc(with_bias)
    in_maps = [
        {
            "prev_a": prev_a[c * BPC : (c + 1) * BPC],
            "filter_w": filter_w,
            "filter_b": filter_b,
        }
        for c in range(N_CORES)
    ]
    trace = os.environ.get("KERNEL_TRACE") == "1"
    res = run_bass_kernel_spmd(nc, in_maps, list(range(N_CORES)), trace=trace)
    LAST_RESULTS = res
    return np.concatenate([res.results[c]["out"] for c in range(N_CORES)], axis=0)


# revision 36
# speedup vs baseline: 1.0079x; 1.0009x over previous
"""3x3 same-conv (NHWC, 32x56x56x128 -> 32x56x56x256) + bias + ReLU on 8 TRN2 cores.

Strategy: data-parallel over batch (4 images/core). Per core, the conv is
9 shifted matmuls accumulated in PSUM with Cin=128 as the contraction dim
over a PACKED 56-wide slab: XpT[cin, p] with p(r,c) = (r+1)*56+c+1, only
vertical pad rows (no left/right pad columns). The anchor axis is then a
dense [57, 3193) range tiled in 25 windows of exactly 128 anchors: every
matmul runs with the full 128 PSUM partitions / 128-column stationary
operand (fast weight load), there are zero junk anchors, and the store is
a handful of large regular DMAs per image ([128, nw, 256] window-major ->
pixel-major DRAM).

The packed layout makes the horizontal taps WRAP at row edges, so output
columns 0 and 55 are recomputed by a small edge pass: column strips are
copied into a column-major mini-slab of three zero-padded regions
([col0|col55] / [col1|zeros] / [zeros|col54]) so NINE full-window M=113
matmuls rebuild both edge columns exactly (no partition-offset PSUM
writes -- those land wrong on HW). The edge stores sit behind the
image's main stores on the same SWDGE ring: FIFO order is the only
cross-DMA write-ordering guarantee (a different queue measured racy).

Input transposes run on the PE (identity-matmul of 2-row stage chunks;
the DMA-XBAR alternative measures ~1.2us/chunk and serializes against
other DMAs). A warm-up burst of identity transposes at t=0 releases the
HAM clock gate before the first real matmul. Input stage loads are
casting SWDGE DMAs (fp32->fp16) issued up front, except the weights and
first stage chunks which ride the faster HWDGE ring as fp32 + DVE cast.
"""

import os
from contextlib import ExitStack

import numpy as np

import concourse.bass as bass
import concourse.bacc as bacc
import concourse.mybir as mybir
import concourse.tile as tile
from concourse.bass_utils import run_bass_kernel_spmd
from concourse.masks import make_identity

N_CORES = 8
B, H, W, CIN, COUT = 32, 56, 56, 128, 256
BPC = B // N_CORES            # images per core
PIX = H * W                   # 3136
SLAB_W = 3328                 # 1 + pad row + 56 rows + pad row + slop
ABASE = W + 1                 # first anchor (pixel (0,0) at 57)
NW = 25                       # 128-anchor windows per image
WM = 128                      # anchors per window
RPC = 2                       # image rows per transpose chunk
CHUNK_PIX = RPC * W           # 112
NCHUNK = H // RPC             # 28
ES = H + 2                    # edge strip length (58)

TAP_OFFS = [(dh - 1) * W + (dw - 1) for dh in range(3) for dw in range(3)]
F32 = mybir.dt.float32
F16 = mybir.dt.float16

LAST_RESULTS = None


def _build(with_bias: bool):
    nc = bacc.Bacc("TRN2", target_bir_lowering=False, debug=False)
    x_h = nc.declare_dram_parameter("prev_a", [BPC, H, W, CIN], F32, isOutput=False)
    w_h = nc.declare_dram_parameter("filter_w", [3, 3, CIN, COUT], F32, isOutput=False)
    b_h = nc.declare_dram_parameter("filter_b", [1, 1, 1, COUT], F32, isOutput=False)
    y_h = nc.declare_dram_parameter("out", [BPC, H, W, COUT], F32, isOutput=True)
    x_ap, w_ap, b_ap, y_ap = x_h.ap(), w_h.ap(), b_h.ap(), y_h.ap()

    with tile.TileContext(nc) as tc, ExitStack() as ctx:
        const_pool = ctx.enter_context(tc.tile_pool(name="const", bufs=1))
        xslab_pool = ctx.enter_context(tc.tile_pool(name="xslab", bufs=1))
        stage_pool = ctx.enter_context(tc.tile_pool(name="stage", bufs=1))
        edge_pool = ctx.enter_context(tc.tile_pool(name="edge", bufs=2))
        out_pool = ctx.enter_context(tc.tile_pool(name="outsb", bufs=2))
        psum_mm = ctx.enter_context(
            tc.tile_pool(name="psmm", bufs=3, space=bass.MemorySpace.PSUM)
        )
        psum_tp = ctx.enter_context(
            tc.tile_pool(name="pstp", bufs=3, space=bass.MemorySpace.PSUM)
        )
        psum_ed = ctx.enter_context(
            tc.tile_pool(name="psed", bufs=2, space=bass.MemorySpace.PSUM)
        )

        identity = const_pool.tile([CHUNK_PIX, CHUNK_PIX], F16, tag="ident")
        make_identity(nc, identity[:])

        # Stage tiles + loads: all four images' [pix, cin] fp16 stages up
        # front (casting SWDGE DMAs); the weight cast rides between the first
        # image's chunks so the first window can start ASAP.
        stages = [
            stage_pool.tile(
                [CHUNK_PIX, NCHUNK * CIN], F16, tag=f"stg{i}", name=f"stg{i}"
            )
            for i in range(BPC)
        ]
        wslab = const_pool.tile([CIN, 9 * COUT], F16, tag="wslab")

        def emit_load(i, c0, c1):
            src = (
                x_ap[i]
                .rearrange("h w c -> (h w) c")
                .rearrange("(n p) c -> n p c", p=CHUNK_PIX)
                .transpose([1, 0, 2])
            )
            dstv = stages[i][:].rearrange("p (n c) -> p n c", n=NCHUNK)
            nc.gpsimd.dma_start(out=dstv[:, c0:c1, :], in_=src[:, c0:c1, :])

        # Weights ride the (otherwise idle) HWDGE queue as fp32 + DVE cast;
        # a casting SWDGE load here would serialize behind the stage loads
        # and gate the first conv matmul (~8us later, measured).
        # Weights and the first stage chunks ride the HWDGE ring as fp32
        # (cast by DVE): the SWDGE casting transfers are too slow to feed
        # the PE early, and a single 1.2MB weights DMA would gate the first
        # conv matmul -- interleave them split in halves.
        wstage = const_pool.tile([CIN, 9 * COUT], F32, tag="wstage")
        wsrc = w_ap.rearrange("a b k n -> (a b) k n").transpose([1, 0, 2])
        wdst = wstage[:].rearrange("k (t n) -> k t n", t=9)
        HS = 6
        stage32 = const_pool.tile([CHUNK_PIX, HS * CIN], F32, tag="stage32")
        hs_src = (
            x_ap[0]
            .rearrange("h w c -> (h w) c")
            .rearrange("(n p) c -> n p c", p=CHUNK_PIX)
            .transpose([1, 0, 2])
        )
        hs_dst = stage32[:].rearrange("p (n c) -> p n c", n=HS)
        st_dst = stages[0][:].rearrange("p (n c) -> p n c", n=NCHUNK)
        nc.sync.dma_start(out=hs_dst[:, 0:2, :], in_=hs_src[:, 0:2, :])
        nc.vector.tensor_copy(st_dst[:, 0:2, :], hs_dst[:, 0:2, :])
        nc.sync.dma_start(out=wdst[:, 0:5, :], in_=wsrc[:, 0:5, :])
        # per-block casts so the first taps leave the critical path early
        nc.vector.tensor_copy(wslab[:, : 2 * COUT], wstage[:, : 2 * COUT])
        nc.vector.tensor_copy(
            wslab[:, 2 * COUT : 5 * COUT], wstage[:, 2 * COUT : 5 * COUT]
        )
        nc.sync.dma_start(out=hs_dst[:, 2:HS, :], in_=hs_src[:, 2:HS, :])
        nc.vector.tensor_copy(st_dst[:, 2:HS, :], hs_dst[:, 2:HS, :])
        nc.sync.dma_start(out=wdst[:, 5:9, :], in_=wsrc[:, 5:9, :])
        nc.vector.tensor_copy(wslab[:, 5 * COUT : 7 * COUT], wstage[:, 5 * COUT : 7 * COUT])
        nc.vector.tensor_copy(wslab[:, 7 * COUT :], wstage[:, 7 * COUT :])

        emit_load(0, HS, 14)
        emit_load(0, 14, 21)
        emit_load(0, 21, 28)
        for i in range(1, BPC):
            for c0 in range(0, NCHUNK, 7):
                emit_load(i, c0, c0 + 7)

        if with_bias:
            bias_st = const_pool.tile([1, COUT], F32, tag="bias_st")
            nc.sync.dma_start(
                out=bias_st[:], in_=b_ap.rearrange("a b c n -> (a b c) n")
            )
            bias_sb = const_pool.tile([1, COUT], F16, tag="bias")
            nc.vector.tensor_copy(bias_sb[:], bias_st[:])
            ones_sb = const_pool.tile([1, 128], F16, tag="ones")
            nc.gpsimd.memset(ones_sb[:], 1.0)

        # PE warm-up: ~30 junk identity transposes keep the PE busy through
        # the HAM activity window (~3.4us) while the first DMAs land, so the
        # first real matmul runs at the warm 2.4 GHz clock.
        for _ in range(22):
            pwu = psum_tp.tile([CIN, CHUNK_PIX], F16, tag="pst")
            nc.tensor.transpose(
                pwu[0:CHUNK_PIX, 0:CHUNK_PIX], identity[:], identity[:]
            )

        # Per-image packed transposed slabs [cin, 1 + 58*56 rows + slop]
        xslabs = []
        for i in range(BPC):
            sl = xslab_pool.tile([CIN, SLAB_W], F16, tag=f"xs{i}")
            xslabs.append(sl)
            nc.vector.memset(sl[:, 0 : ABASE], 0.0)           # lead + top pad row
            nc.vector.memset(sl[:, (H + 1) * W + 1 : SLAB_W], 0.0)  # bottom pad + slop

        def emit_transpose(i, cidx):
            # stage [112, cin] chunk -> PE transpose -> PSUM [cin, 112] ->
            # ACT copy into the packed slab (contiguous 112 span)
            pst = psum_tp.tile([CIN, CHUNK_PIX], F16, tag="pst")
            nc.tensor.transpose(
                pst[:], stages[i][:, cidx * CIN : (cidx + 1) * CIN], identity[:]
            )
            d0 = (RPC * cidx + 1) * W + 1
            nc.scalar.activation(
                xslabs[i][:, d0 : d0 + CHUNK_PIX],
                pst[:],
                mybir.ActivationFunctionType.Copy,
            )

        # edge mini-slab: three 115-wide regions [pad a(56) sharedpad b(56)
        # pad] = A:[col0|col55] B:[col1|zeros] C:[zeros|col54], so all nine
        # edge matmuls are FULL M=113 windows (partitions 0-55 = left col,
        # 57-112 = right col; the zero halves contribute nothing) -- no
        # partition-offset PSUM writes, which measure wrong on HW.
        RA, RB, RC = 0, 115, 230
        EW = 352
        EM = 113
        ECOPY = [(RA + 1, 0), (RA + 58, 55), (RB + 1, 1), (RC + 58, 54)]

        def emit_ebuild(i):
            ed = edge_pool.tile([CIN, EW], F16, tag="E")
            nc.vector.memset(ed[:], 0.0)
            for base, col in ECOPY:
                src = (
                    xslabs[i][:, W + 1 + col : W + 1 + col + H * W]
                    .rearrange("p (r c) -> p r c", c=W)[:, :, 0:1]
                    .rearrange("p r c -> p (r c)")
                )
                nc.scalar.activation(
                    ed[:, base : base + H],
                    src,
                    mybir.ActivationFunctionType.Copy,
                )
            return ed

        def emit_window(i, w, oslab):
            q0 = ABASE + WM * w
            ps = psum_mm.tile([WM, COUT], F32, tag="psmm")
            for t in range(9):
                w0 = q0 + TAP_OFFS[t]
                nc.tensor.matmul(
                    ps[:],
                    xslabs[i][:, w0 : w0 + WM],
                    wslab[:, t * COUT : (t + 1) * COUT],
                    start=(t == 0),
                    stop=(t == 8 and not with_bias),
                )
            if with_bias:
                nc.tensor.matmul(
                    ps[:], ones_sb[:1, :WM], bias_sb[:1, :], start=False, stop=True
                )
            nc.vector.tensor_scalar_max(
                oslab[:, w * COUT : (w + 1) * COUT], ps[:], 0.0
            )

        # main-store chunk boundaries (after these windows' relu); finer at
        # the image end so the final transfer (which the edge stores
        # WAW-wait on) is small
        STORE_AT = {
            6: (0, 7),
            13: (7, 14),
            19: (14, 20),
            20: (20, 21),
            21: (21, 22),
            22: (22, 23),
            23: (23, 24),
        }

        def emit_store_chunk(i, oslab, w0, w1):
            dst = (
                y_ap[i]
                .rearrange("h w c -> (h w) c")[w0 * WM : w1 * WM, :]
                .rearrange("(w p) c -> p w c", p=WM)
            )
            src = oslab[:, w0 * COUT : w1 * COUT].rearrange(
                "p (w k) -> p w k", k=COUT
            )
            nc.gpsimd.dma_start(out=dst, in_=src)

        def emit_store_last(i, oslab):
            n = PIX - 24 * WM  # 64
            dst = y_ap[i].rearrange("h w c -> (h w) c")[24 * WM :, :]
            nc.gpsimd.dma_start(out=dst, in_=oslab[0:n, 24 * COUT : 25 * COUT])

        # edge pass: 9 full-window matmuls rebuild output cols 0 / 55
        # exactly; their stores overwrite the wrapped main-store values and
        # MUST sit behind the image's main stores on the same SWDGE queue --
        # FIFO ring order is the only cross-DMA write-ordering guarantee
        # (stores on a different queue measured racy: ~1-2 columns stale).
        E_REGION = {0: RC, 1: RA, 2: RB}  # dw -> region base

        def emit_edge(i, ed):
            pe = psum_ed.tile([EM, COUT], F32, tag="psed")
            k = 0
            nmm = 9 + (1 if with_bias else 0)
            for dh in range(3):
                for dw in range(3):
                    t = dh * 3 + dw
                    base = E_REGION[dw]
                    nc.tensor.matmul(
                        pe[:],
                        ed[:, base + dh : base + dh + EM],
                        wslab[:, t * COUT : (t + 1) * COUT],
                        start=(k == 0),
                        stop=(k == nmm - 1),
                    )
                    k += 1
            if with_bias:
                nc.tensor.matmul(
                    pe[:], ones_sb[:1, :EM], bias_sb[:1, :], start=False, stop=True
                )
            esb = edge_pool.tile([EM, COUT], F32, tag="esb")
            nc.vector.tensor_scalar_max(esb[:], pe[:], 0.0)
            return esb

        def emit_edge_stores(i, esb):
            nc.gpsimd.dma_start(out=y_ap[i][:, 0, :], in_=esb[0:H, :])
            nc.gpsimd.dma_start(out=y_ap[i][:, 55, :], in_=esb[H + 1 : H + 1 + H, :])

        # Image 0's transposes up front; image i+1's are interleaved between
        # image i's windows so the PE never waits on a bulk transpose phase.
        for c in range(NCHUNK):
            emit_transpose(0, c)
        ed = emit_ebuild(0)
        for i in range(BPC):
            oslab = out_pool.tile([WM, NW * COUT], F32, tag="osb")
            done = 0
            for w in range(NW):
                emit_window(i, w, oslab)
                if w in STORE_AT:
                    emit_store_chunk(i, oslab, *STORE_AT[w])
                if w == 21:
                    esb = emit_edge(i, ed)
                if i + 1 < BPC:
                    want = (w + 1) * NCHUNK // NW
                    while done < want:
                        emit_transpose(i + 1, done)
                        done += 1
            emit_store_last(i, oslab)
            emit_edge_stores(i, esb)
            if i + 1 < BPC:
                while done < NCHUNK:
                    emit_transpose(i + 1, done)
                    done += 1
            ed = emit_ebuild(i + 1) if i + 1 < BPC else None

    nc.compile()
    return nc


_CACHE = {}


def _get_nc(with_bias: bool):
    if with_bias not in _CACHE:
        _CACHE[with_bias] = _build(with_bias)
    return _CACHE[with_bias]


def kernel(prev_a, filter_w, filter_b):
    global LAST_RESULTS
    prev_a = np.ascontiguousarray(prev_a, dtype=np.float32)
    filter_w = np.ascontiguousarray(filter_w, dtype=np.float32)
    filter_b = np.ascontiguousarray(filter_b, dtype=np.float32).reshape(1, 1, 1, COUT)
    with_bias = bool(np.any(filter_b))
    nc = _get_nc(with_bias)
    in_maps = [
        {
            "prev_a": prev_a[c * BPC : (c + 1) * BPC],
            "filter_w": filter_w,
            "filter_b": filter_b,
        }
        for c in range(N_CORES)
    ]
    trace = os.environ.get("KERNEL_TRACE") == "1"
    res = run_bass_kernel_spmd(nc, in_maps, list(range(N_CORES)), trace=trace)
    LAST_RESULTS = res
    return np.concatenate([res.results[c]["out"] for c in range(N_CORES)], axis=0)
